# revision 41
# baseline (speedup 1.0000x reference)
"""Causal self-attention (B=2, T=2048, C=1024, H=16, D=64) on 8 trn2 cores.

Sharding: tensor-parallel on heads — 2 heads per core. Each core computes
QKV projection for its 2 heads, causal softmax attention, and its heads'
slice of the output projection (a rank-128 partial sum of the full output).
The host pre-transposes x to [B, C, T], slices the weights per core, and
sums the 8 partial outputs (+ proj bias) at the end.

v2 changes over the bf16 baseline (206us):
  - QKV projection in fp8e4 with DoubleRow (K=256 per pass): x^T and the
    qkv weights ship as e4m3 (weights pre-scaled by 64 on the host; the
    1/64 is folded into the psum->sbuf bias-add, which moved ACT->DVE
    tensor_scalar to keep the scalar engine free for exp).
  - O accumulation in fp8e4 DoubleRow: exp writes P^T directly as e4m3
    into [128, 2, QT] pair tiles; V is transposed into a [128, 16, 2, 72]
    token-major layout (ones column at 64 gives the softmax denominator),
    and each O matmul contracts a 256-deep pair of k-chunks.
  - PE warmup accumulation chain + early exp-table load at t=0 so HAM
    reaches K=8/8 before real work and the first exp doesn't stall.
  - Output partials stored as bf16 (host sums in float64).
  - Filler queue is cost-weighted; attention pops ~600ns of dense PE work
    per k-chunk to keep the PE streaming while ACT runs exp wall-to-wall.
"""

from collections import deque

import numpy as np

import concourse.bass as bass
import concourse.tile as tile
from concourse import bacc, mybir
from concourse.bass_utils import run_bass_kernel_spmd

dt = mybir.dt
AF = mybir.ActivationFunctionType
DR = mybir.MatmulPerfMode.DoubleRow

B, T, C, H, D = 2, 2048, 1024, 16, 64
NCORES = 8
HPC = H // NCORES          # heads per core = 2
QT = 1024                  # q-tile (columns of S^T/O^T psum tiles)
KC = 128                   # k chunk (partition dim of S^T)
KP = 256                   # k pair (DoubleRow O contraction)
SUB = 512                  # psum bank subtile (fp32)
SCALE = 1.0 / 8.0          # 1/sqrt(D)
WSCL = 64.0                # host-side qkv weight scale (fp8 range)
QKV_FP8 = True             # fp8 DoubleRow QKV projection
O_FP8 = True               # fp8 DoubleRow O accumulation

_CACHE = {}


def _emit(tc):
    from contextlib import ExitStack
    with ExitStack() as ctx:
        _emit_body(tc, ctx)


def _emit_body(tc, ctx):
    nc = tc.nc
    f32, bf16, f8 = dt.float32, dt.bfloat16, dt.float8e4

    xT = nc.dram_tensor("xT", [B, C, T], f8 if QKV_FP8 else bf16,
                        kind="ExternalInput").ap()
    xTe = nc.dram_tensor("xTe", [B, C, 256], bf16, kind="ExternalInput").ap()
    wqkv = nc.dram_tensor("wqkv", [C, 384], f8, kind="ExternalInput").ap()
    wqkv16 = nc.dram_tensor("wqkv16", [C, 384], bf16,
                            kind="ExternalInput").ap()
    bqkv = nc.dram_tensor("bqkv", [128, 3], f32, kind="ExternalInput").ap()
    wp = nc.dram_tensor("wp", [128, C], bf16, kind="ExternalInput").ap()
    dmsk = nc.dram_tensor("dmsk", [128, 2, 256], f8, kind="ExternalInput").ap()
    ident = nc.dram_tensor("ident", [128, 128], bf16, kind="ExternalInput").ap()
    wrm = nc.dram_tensor("wrm", [128, 512], bf16, kind="ExternalInput").ap()
    outp = nc.dram_tensor("outp", [B, T, C], bf16, kind="ExternalOutput").ap()
    wsink = nc.dram_tensor("wsink", [128, 528], bf16, kind="ExternalOutput").ap()

    consts = ctx.enter_context(tc.tile_pool(name="consts", bufs=1))
    xpool = ctx.enter_context(tc.tile_pool(name="xpool", bufs=2))
    qkvpool = ctx.enter_context(tc.tile_pool(name="qkvpool", bufs=6))
    vtmpool = ctx.enter_context(tc.tile_pool(name="vtmpool", bufs=2))
    ptpool = ctx.enter_context(tc.tile_pool(name="ptpool", bufs=6))
    ptbpool = ctx.enter_context(tc.tile_pool(name="ptbpool", bufs=2))
    unormp = ctx.enter_context(tc.tile_pool(name="unormp", bufs=3))
    rows = ctx.enter_context(tc.tile_pool(name="rows", bufs=2))
    outsb = ctx.enter_context(tc.tile_pool(name="outsb", bufs=8))
    warmsb = ctx.enter_context(tc.tile_pool(name="warmsb", bufs=1))
    stp = ctx.enter_context(tc.tile_pool(name="stp", bufs=2, space="PSUM"))
    otp = ctx.enter_context(tc.tile_pool(name="otp", bufs=1, space="PSUM"))
    miscp = ctx.enter_context(tc.tile_pool(name="miscp", bufs=2, space="PSUM"))

    # constants / weights resident in SBUF. DMA order is load-bearing: the
    # sync queue serializes transfers, so batch-0 x chunks must not sit
    # behind const tensors that are only needed later (w16/wp/dmsk).
    # Batch-1 x ships on the scalar engine's queue, in parallel.
    id_sb = consts.tile([128, 128], bf16, tag="id")
    nc.sync.dma_start(out=id_sb, in_=ident)
    wrm_sb = consts.tile([128, 512], bf16, tag="wrm")
    nc.sync.dma_start(out=wrm_sb, in_=wrm)
    w_sb = consts.tile([128, 8, 384], f8, tag="w")
    nc.sync.dma_start(out=w_sb, in_=wqkv.rearrange("(k p) f -> p k f", p=128))
    b_sb = consts.tile([128, 3], f32, tag="b")
    nc.sync.dma_start(out=b_sb, in_=bqkv)

    # --- warmup: ~10 N=512 matmuls (one accumulation chain so nothing is
    # dead) + a tiny exp to pull the ACT table load off the critical path.
    warm_ps = miscp.tile([128, SUB], f32, tag="misc", name="warm")
    for i in range(10):
        nc.tensor.matmul(warm_ps[:, :], id_sb[:, :], wrm_sb[:, :],
                         start=(i == 0), stop=(i == 9))
    warm_out = warmsb.tile([128, 528], bf16, tag="wo")
    nc.scalar.activation(warm_out[:, 512:528], id_sb[:, 0:16], AF.Exp)
    nc.vector.tensor_copy(warm_out[:, 0:512], warm_ps[:, :])
    nc.sync.dma_start(out=wsink, in_=warm_out)

    # early-attention deps go first on the scalar queue, before batch-1 x
    w16_sb = consts.tile([128, 8, 384], bf16, tag="w16")
    nc.scalar.dma_start(out=w16_sb,
                        in_=wqkv16.rearrange("(k p) f -> p k f", p=128))
    dmsk_sb = consts.tile([128, 2, 256], f8, tag="dmsk")
    nc.scalar.dma_start(out=dmsk_sb, in_=dmsk)

    # x^T: batch 0 on sync queue, batch 1 on scalar queue (parallel)
    xps, xes = [], []
    for b in range(B):
        eng = nc.sync if b == 0 else nc.scalar
        xp = xpool.tile([128, 8, T], f8 if QKV_FP8 else bf16, tag="xp",
                        name=f"xp{b}")
        xsrc = xT[b].rearrange("(j p) t -> p j t", p=128)
        xe = xpool.tile([128, 8, 256], bf16, tag="xe", name=f"xe{b}")
        eng.dma_start(out=xe, in_=xTe[b].rearrange(
            "(j p) t -> p j t", p=128))
        xes.append(xe)
        for tg in range(T // SUB):
            t0 = tg * SUB
            eng.dma_start(out=xp[:, :, t0:t0 + SUB],
                          in_=xsrc[:, :, t0:t0 + SUB])
        xps.append(xp)

    # later-needed consts after batch-0 x
    wp_sb = consts.tile([128, C], bf16, tag="wp")
    nc.sync.dma_start(out=wp_sb, in_=wp)

    filler = deque()
    fstate = {"cost": 0, "pops_left": 96}  # 96 attention chunk-pops total

    def fpush(thunks):
        filler.extend(thunks)
        fstate["cost"] += sum(c for c, _ in thunks)

    def pop_filler(budget=None):
        # spread the remaining filler evenly over the remaining attention
        # chunks so the PE neither starves late nor hoards early
        if budget is None:
            left = max(1, fstate["pops_left"])
            budget = max(500, fstate["cost"] // left)
        while filler and budget > 0:
            cost, th = filler.popleft()
            fstate["cost"] -= cost
            th()
            budget -= cost

    def make_qkv(b):
        """qkvT tiles + thunks per (tg, m, n): 4-MM fp8-DR chains.

        Returns (dsts, front, rest): `front` covers tokens 0-1023 plus the
        bf16 early-token fix (everything q-tile 0 attention needs); `rest`
        is the tg=1 half, safe to run as attention filler."""
        dsts = [qkvpool.tile([128, T], bf16, tag="qkv", name=f"qkv{b}_{m}")
                for m in range(3)]
        by_tg = {0: [], 1: []}
        for tg in range(T // 1024):
            for m in range(3):
                for n in range(2):
                    def th(m=m, tg=tg, n=n):
                        t0 = tg * 1024 + n * SUB
                        pg = miscp.tile([128, SUB], f32, tag="misc",
                                        name="pg")
                        if QKV_FP8:
                            for c in range(4):
                                nc.tensor.matmul(
                                    pg[:, :],
                                    w_sb[:, 2 * c:2 * c + 2,
                                         128 * m:128 * m + 128],
                                    xps[b][:, 2 * c:2 * c + 2, t0:t0 + SUB],
                                    start=(c == 0), stop=(c == 3),
                                    perf_mode=DR,
                                )
                            nc.vector.tensor_scalar(
                                out=dsts[m][:, t0:t0 + SUB], in0=pg[:, :],
                                scalar1=1.0 / WSCL, scalar2=b_sb[:, m:m + 1],
                                op0=mybir.AluOpType.mult,
                                op1=mybir.AluOpType.add)
                            pass
                        else:
                            for kc in range(8):
                                nc.tensor.matmul(
                                    pg[:, :],
                                    w16_sb[:, kc, 128 * m:128 * m + 128],
                                    xps[b][:, kc, t0:t0 + SUB],
                                    start=(kc == 0), stop=(kc == 7),
                                )
                            nc.vector.tensor_scalar(
                                out=dsts[m][:, t0:t0 + SUB], in0=pg[:, :],
                                scalar1=b_sb[:, m:m + 1], scalar2=None,
                                op0=mybir.AluOpType.add)
                    by_tg[tg].append((1150, th))
        # tokens 0-255 recomputed in bf16: softmax rows with few valid keys
        # amplify fp8 noise, so the early tokens' q/k/v must be clean.
        fix = []
        if QKV_FP8:
            for m in range(3):
                def thfix(m=m):
                    pg = miscp.tile([128, 256], f32, tag="misc", name="pgf")
                    for kc in range(8):
                        nc.tensor.matmul(
                            pg[:, :],
                            w16_sb[:, kc, 128 * m:128 * m + 128],
                            xes[b][:, kc, :],
                            start=(kc == 0), stop=(kc == 7),
                        )
                    nc.vector.tensor_scalar(
                        out=dsts[m][:, 0:256], in0=pg[:, :],
                        scalar1=b_sb[:, m:m + 1], scalar2=None,
                        op0=mybir.AluOpType.add)
                fix.append((1000, thfix))
        return dsts, by_tg[0] + fix, by_tg[1]

    def make_vt(b, vT_t):
        """V to token-major [128, 16, 2, 72] fp8 with ones cols; 9 thunks.

        Chunks 0/1 (k < 256) are additionally kept in bf16 (vtb) for the
        precision-critical first O pair of q-tile 0."""
        vt = vtmpool.tile([128, 16, HPC, 72], f8 if O_FP8 else bf16,
                          tag="vtm", name=f"vt{b}")
        vtb = vtmpool.tile([128, 2, HPC, 72], bf16, tag="vtb", name=f"vtb{b}")

        def th0():
            nc.vector.memset(vt[:, :, :, 64:65], 1.0)
            nc.vector.memset(vtb[:, :, :, 64:65], 1.0)
        thunks = [(150, th0)]
        for j0 in range(0, T // 128, 2):
            def th(j0=j0):
                for j in (j0, j0 + 1):
                    tp = miscp.tile([128, 128], bf16, tag="misc", name="tp")
                    nc.tensor.transpose(
                        tp[:, :], vT_t[:, 128 * j:128 * j + 128], id_sb[:, :])
                    nc.vector.tensor_copy(
                        out=vt[:, j, :, 0:64],
                        in_=tp.rearrange("p (h c) -> p h c", h=HPC),
                    )
                    if j < 2:
                        nc.vector.tensor_copy(
                            out=vtb[:, j, :, 0:64],
                            in_=tp.rearrange("p (h c) -> p h c", h=HPC),
                        )
            thunks.append((650, th))
        return (vt, vtb), thunks

    def make_proj(b, q0, un, trange):
        """Projection thunks for q-rows trange of one q-tile."""
        thunks = []
        for ts in trange:
            for ct in range(C // SUB):
                def th(ts=ts, ct=ct):
                    a0 = q0 + ts * 128
                    pp = miscp.tile([128, SUB], f32, tag="misc", name="pp")
                    nc.tensor.matmul(
                        pp[:, :],
                        un[:, ts * 128:(ts + 1) * 128],
                        wp_sb[:, ct * SUB:(ct + 1) * SUB],
                        start=True, stop=True,
                    )
                    ob = outsb.tile([128, SUB], bf16, tag="osb")
                    nc.any.tensor_copy(ob[:, :], pp[:, :])
                    nc.sync.dma_start(
                        out=outp[b, a0:a0 + 128, ct * SUB:(ct + 1) * SUB],
                        in_=ob[:, :])
                thunks.append((450, th))
        return thunks

    # batch 0 front work: only what q-tile-0 attention needs runs densely
    # (QKV tokens 0-1023 + fix + V chunks 0-7); the rest becomes filler.
    qkv0, front0, rest0 = make_qkv(0)
    for _, th in front0:
        th()
    vt0, vth0 = make_vt(0, qkv0[2])
    for _, th in vth0[:5]:
        th()

    qkv_t, vt_t = {0: qkv0}, {0: vt0}

    for b in range(B):
        if b == 0:
            # queue the rest of the front work + batch 1 as attention filler
            qkv1, front1, rest1 = make_qkv(1)
            vt1, vth1 = make_vt(1, qkv1[2])
            fpush(rest0)
            fpush(vth0[5:])
            fpush(front1)
            fpush(rest1)
            fpush(vth1)
            qkv_t[1], vt_t[1] = qkv1, vt1
        qT_t, kT_t, vT_t = qkv_t[b]
        vt, vtb = vt_t[b]

        for qt in range(T // QT):
            q0 = qt * QT
            npair = (q0 + QT) // KP
            un = unormp.tile([128, QT], bf16, tag="un", name=f"un{b}{qt}")
            undone = [0, 0]  # per-half: heads whose norm is emitted
            for h in range(HPC):
                qT_h = qT_t[64 * h:64 * h + 64, :]
                kT_h = kT_t[64 * h:64 * h + 64, :]
                ot = otp.tile([65, QT], f32, tag="ot")

                def emit_o(p, pt2):
                    """O^T accumulate for k-pair p: fp8 DoubleRow, K=256."""
                    lsp = max(0, p * KP - q0)
                    diag = p * KP >= q0
                    vpair = vt[:, 2 * p:2 * p + 2, h, 0:65]
                    for n in range(QT // SUB):
                        s0 = max(n * SUB, lsp)
                        if s0 >= (n + 1) * SUB:
                            continue
                        if diag and s0 == lsp:
                            s0 = lsp + KP  # masked region emitted separately
                            if s0 >= (n + 1) * SUB:
                                continue
                        last_p = (q0 + (n + 1) * SUB) // KP - 1
                        nc.tensor.matmul(
                            ot[:, s0:(n + 1) * SUB],
                            vpair,
                            pt2[:, :, s0:(n + 1) * SUB],
                            start=(p == 0 and q0 > 0), stop=(p == last_p),
                            perf_mode=DR,
                        )
                    if diag:
                        # region already started by p=0's full-subtile MM
                        n0 = lsp // SUB
                        last_p = (q0 + (n0 + 1) * SUB) // KP - 1
                        nc.tensor.matmul(
                            ot[:, lsp:lsp + KP],
                            vpair,
                            pt2[:, :, lsp:lsp + KP],
                            start=False, stop=(p == last_p),
                            perf_mode=DR,
                        )

                def emit_o_chunk(kc, ptj, vsrc):
                    """Per-chunk O^T accumulate (K=128, non-DR)."""
                    ls = max(0, kc * KC - q0)
                    diag = kc * KC >= q0
                    for n in range(QT // SUB):
                        s0 = max(n * SUB, ls)
                        if s0 >= (n + 1) * SUB:
                            continue
                        if diag and s0 == ls:
                            s0 = ls + 128
                            if s0 >= (n + 1) * SUB:
                                continue
                        last_kc = (q0 + (n + 1) * SUB) // KC - 1
                        nc.tensor.matmul(
                            ot[:, s0:(n + 1) * SUB], vsrc,
                            ptj[:, s0:(n + 1) * SUB],
                            start=(kc == 0), stop=(kc == last_kc),
                        )
                    if diag:
                        n0 = ls // SUB
                        last_kc = (q0 + (n0 + 1) * SUB) // KC - 1
                        nc.tensor.matmul(
                            ot[:, ls:ls + 128], vsrc, ptj[:, ls:ls + 128],
                            start=False, stop=(kc == last_kc),
                        )

                def norm_half(half):
                    """Normalize cols [half*SUB, (half+1)*SUB) of this head
                    into un as soon as their O accumulation completes."""
                    c0 = half * SUB
                    se = rows.tile([1, SUB], f32, tag="se", name=f"se{h}")
                    nc.vector.tensor_copy(se[:, :], ot[64:65, c0:c0 + SUB])
                    rc = rows.tile([1, SUB], f32, tag="rc", name=f"rc{h}")
                    nc.vector.reciprocal_approx_fast(rc[:, :], se[:, :])
                    rb = rows.tile([64, SUB], f32, tag="rb", name=f"rb{h}")
                    nc.gpsimd.partition_broadcast(rb[:, :], rc[:, :])
                    nc.vector.tensor_mul(
                        un[64 * h:64 * h + 64, c0:c0 + SUB],
                        ot[0:64, c0:c0 + SUB], rb[:, :])
                    undone[half] += 1
                    if undone[half] == HPC:
                        # both heads done: this half's proj can go out
                        fpush(make_proj(
                            b, q0, un, range(4 * half, 4 * half + 4)))

                for p in range(npair):
                    lsp = max(0, p * KP - q0)
                    diag = p * KP >= q0
                    bf_pair = O_FP8 and (q0 == 0 and p == 0)
                    if bf_pair:
                        pt2 = ptbpool.tile([128, 2, QT], bf16, tag="ptb",
                                           name="ptb")
                    else:
                        pt2 = ptpool.tile([128, 2, QT], f8 if O_FP8 else bf16,
                                          tag="pt")
                    for j in range(2):  # the two k-chunks of the pair
                        kc = 2 * p + j
                        k0 = kc * KC
                        ls = max(0, k0 - q0)
                        st = stp.tile([128, QT], f32, tag="st")
                        for n in range(QT // SUB):
                            s0 = max(n * SUB, ls)
                            if s0 >= (n + 1) * SUB:
                                continue
                            nc.tensor.matmul(
                                st[:, s0:(n + 1) * SUB],
                                kT_h[:, k0:k0 + KC],
                                qT_h[:, q0 + s0:q0 + (n + 1) * SUB],
                                start=True, stop=True,
                            )
                        nc.scalar.activation(
                            pt2[:, j, ls:QT], st[:, ls:QT], AF.Exp,
                            scale=SCALE)
                        if not (b == 0 and qt == 0 and p == 0):
                            pop_filler(600)
                        fstate["pops_left"] -= 1
                    if diag:
                        # one op on the (otherwise idle) gpsimd engine: zero
                        # the odd chunk's dead strip + mask both 128-wide
                        # diagonal triangles (memset only initializes the
                        # never-written strip for the mask's read)
                        nc.vector.memset(pt2[:, 1, lsp:lsp + 128], 0.0)
                        nc.vector.tensor_mul(
                            pt2[:, :, lsp:lsp + KP], pt2[:, :, lsp:lsp + KP],
                            dmsk_sb[:, :, :])
                    if not O_FP8:
                        for j in range(2):
                            kc = 2 * p + j
                            emit_o_chunk(kc, pt2[:, j], vt[:, kc, h, 0:65])
                    elif bf_pair:
                        for j in range(2):
                            emit_o_chunk(j, pt2[:, j], vtb[:, j, h, 0:65])
                    else:
                        emit_o(p, pt2)
                    # completed column halves can normalize immediately
                    if (q0 + (p + 1) * KP) % SUB == 0:
                        half = ((p + 1) * KP - q0) // SUB - 1
                        if 0 <= half < 2:
                            norm_half(half)

    pop_filler(10**9)



def build():
    if "nc" in _CACHE:
        return _CACHE["nc"]
    nc = bacc.Bacc("TRN2", target_bir_lowering=False, debug=False,
                   num_devices=NCORES)
    with tile.TileContext(nc) as tc:
        _emit(tc)
    nc.compile()
    _CACHE["nc"] = nc
    return nc


def make_in_maps(x, qkv_w, qkv_b, proj_w):
    import ml_dtypes
    bf16 = ml_dtypes.bfloat16
    f8 = ml_dtypes.float8_e4m3
    x = np.asarray(x, dtype=np.float32)
    qkv_w = np.asarray(qkv_w, dtype=np.float32)
    qkv_b = np.asarray(qkv_b, dtype=np.float32)
    proj_w = np.asarray(proj_w, dtype=np.float32)

    xTf = np.ascontiguousarray(x.transpose(0, 2, 1))
    xT = xTf.astype(f8 if QKV_FP8 else bf16)
    xTe = np.ascontiguousarray(xTf[:, :, 0:256]).astype(bf16)
    # diag-pair mask [128, 2, 256]: even chunk = [tril | ones],
    # odd chunk = [zeros | tril]
    tri = (np.arange(128)[None, :] >= np.arange(128)[:, None])
    dmsk = np.zeros((128, 2, 256), dtype=np.float32)
    dmsk[:, 0, 0:128] = tri
    dmsk[:, 0, 128:256] = 1.0
    dmsk[:, 1, 128:256] = tri
    dmsk = dmsk.astype(f8)
    ident = np.eye(128, dtype=bf16)
    wrm = np.zeros((128, 512), dtype=bf16)

    in_maps = []
    for c in range(NCORES):
        s = 64 * HPC * c  # first feature row of this core's heads
        wq = qkv_w[:, s:s + 128]
        wk = qkv_w[:, C + s:C + s + 128]
        wv = qkv_w[:, 2 * C + s:2 * C + s + 128]
        wqkv_cat = np.concatenate([wq, wk, wv], axis=1)
        wqkv_c = np.ascontiguousarray(wqkv_cat * WSCL).astype(f8)
        wqkv16_c = np.ascontiguousarray(wqkv_cat).astype(bf16)
        bqkv_c = np.ascontiguousarray(np.stack(
            [qkv_b[s:s + 128], qkv_b[C + s:C + s + 128],
             qkv_b[2 * C + s:2 * C + s + 128]], axis=1))
        wp_c = np.ascontiguousarray(proj_w[s:s + 128, :]).astype(bf16)
        in_maps.append({
            "xT": xT, "xTe": xTe, "wqkv": wqkv_c, "wqkv16": wqkv16_c,
            "bqkv": bqkv_c, "wp": wp_c,
            "dmsk": dmsk, "ident": ident, "wrm": wrm,
        })
    return in_maps


def kernel(x, qkv_w, qkv_b, proj_w, proj_b, _trace=False):
    nc = build()
    in_maps = make_in_maps(x, qkv_w, qkv_b, proj_w)
    res = run_bass_kernel_spmd(nc, in_maps, core_ids=list(range(NCORES)),
                               trace=_trace)
    acc = np.zeros((B, T, C), dtype=np.float64)
    for c in range(NCORES):
        acc += np.asarray(res.results[c]["outp"]).astype(np.float64)
    acc += np.asarray(proj_b, dtype=np.float64)
    out = acc.astype(np.float32)
    _CACHE["last_results"] = res
    return out


# revision 43
# speedup vs baseline: 1.0173x; 1.0173x over previous
"""Causal self-attention (B=2, T=2048, C=1024, H=16, D=64) on 8 trn2 cores.

Sharding: tensor-parallel on heads — 2 heads per core. Each core computes
QKV projection for its 2 heads, causal softmax attention, and its heads'
slice of the output projection (a rank-128 partial sum of the full output).
The host pre-transposes x to [B, C, T], slices the weights per core, and
sums the 8 partial outputs (+ proj bias) at the end.

v2 changes over the bf16 baseline (206us):
  - QKV projection in fp8e4 with DoubleRow (K=256 per pass): x^T and the
    qkv weights ship as e4m3 (weights pre-scaled by 64 on the host; the
    1/64 is folded into the psum->sbuf bias-add, which moved ACT->DVE
    tensor_scalar to keep the scalar engine free for exp).
  - O accumulation in fp8e4 DoubleRow: exp writes P^T directly as e4m3
    into [128, 2, QT] pair tiles; V is transposed into a [128, 16, 2, 72]
    token-major layout (ones column at 64 gives the softmax denominator),
    and each O matmul contracts a 256-deep pair of k-chunks.
  - PE warmup accumulation chain + early exp-table load at t=0 so HAM
    reaches K=8/8 before real work and the first exp doesn't stall.
  - Output partials stored as bf16 (host sums in float64).
  - Filler queue is cost-weighted; attention pops ~600ns of dense PE work
    per k-chunk to keep the PE streaming while ACT runs exp wall-to-wall.
"""

from collections import deque

import numpy as np

import concourse.bass as bass
import concourse.tile as tile
from concourse import bacc, mybir
from concourse.bass_utils import run_bass_kernel_spmd

dt = mybir.dt
AF = mybir.ActivationFunctionType
DR = mybir.MatmulPerfMode.DoubleRow

B, T, C, H, D = 2, 2048, 1024, 16, 64
NCORES = 8
HPC = H // NCORES          # heads per core = 2
QT = 1024                  # q-tile (columns of S^T/O^T psum tiles)
KC = 128                   # k chunk (partition dim of S^T)
KP = 256                   # k pair (DoubleRow O contraction)
SUB = 512                  # psum bank subtile (fp32)
SCALE = 1.0 / 8.0          # 1/sqrt(D)
WSCL = 64.0                # host-side qkv weight scale (fp8 range)
QKV_FP8 = True             # fp8 DoubleRow QKV projection
O_FP8 = True               # fp8 DoubleRow O accumulation

_CACHE = {}


def _emit(tc):
    from contextlib import ExitStack
    with ExitStack() as ctx:
        _emit_body(tc, ctx)


def _emit_body(tc, ctx):
    nc = tc.nc
    f32, bf16, f8 = dt.float32, dt.bfloat16, dt.float8e4

    xT = nc.dram_tensor("xT", [B, C, T], f8 if QKV_FP8 else bf16,
                        kind="ExternalInput").ap()
    xTe = nc.dram_tensor("xTe", [B, C, 256], bf16, kind="ExternalInput").ap()
    wqkv = nc.dram_tensor("wqkv", [C, 384], f8, kind="ExternalInput").ap()
    wqkv16 = nc.dram_tensor("wqkv16", [C, 384], bf16,
                            kind="ExternalInput").ap()
    bqkv = nc.dram_tensor("bqkv", [128, 3], f32, kind="ExternalInput").ap()
    wp = nc.dram_tensor("wp", [128, C], bf16, kind="ExternalInput").ap()
    dmsk = nc.dram_tensor("dmsk", [128, 2, 256], f8, kind="ExternalInput").ap()
    ident = nc.dram_tensor("ident", [128, 128], bf16, kind="ExternalInput").ap()
    wrm = nc.dram_tensor("wrm", [128, 512], bf16, kind="ExternalInput").ap()
    outp = nc.dram_tensor("outp", [B, T, C], bf16, kind="ExternalOutput").ap()
    wsink = nc.dram_tensor("wsink", [128, 528], bf16, kind="ExternalOutput").ap()

    consts = ctx.enter_context(tc.tile_pool(name="consts", bufs=1))
    xpool = ctx.enter_context(tc.tile_pool(name="xpool", bufs=2))
    qkvpool = ctx.enter_context(tc.tile_pool(name="qkvpool", bufs=6))
    vtmpool = ctx.enter_context(tc.tile_pool(name="vtmpool", bufs=2))
    ptpool = ctx.enter_context(tc.tile_pool(name="ptpool", bufs=12))
    ptbpool = ctx.enter_context(tc.tile_pool(name="ptbpool", bufs=2))
    unormp = ctx.enter_context(tc.tile_pool(name="unormp", bufs=3))
    rows = ctx.enter_context(tc.tile_pool(name="rows", bufs=4))
    outsb = ctx.enter_context(tc.tile_pool(name="outsb", bufs=8))
    warmsb = ctx.enter_context(tc.tile_pool(name="warmsb", bufs=1))
    stp = ctx.enter_context(tc.tile_pool(name="stp", bufs=2, space="PSUM"))
    otp = ctx.enter_context(tc.tile_pool(name="otp", bufs=1, space="PSUM"))
    miscp = ctx.enter_context(tc.tile_pool(name="miscp", bufs=2, space="PSUM"))

    # constants / weights resident in SBUF. DMA order is load-bearing: the
    # sync queue serializes transfers, so batch-0 x chunks must not sit
    # behind const tensors that are only needed later (w16/wp/dmsk).
    # Batch-1 x ships on the scalar engine's queue, in parallel.
    id_sb = consts.tile([128, 128], bf16, tag="id")
    nc.sync.dma_start(out=id_sb, in_=ident)
    wrm_sb = consts.tile([128, 512], bf16, tag="wrm")
    nc.sync.dma_start(out=wrm_sb, in_=wrm)
    w_sb = consts.tile([128, 8, 384], f8, tag="w")
    nc.sync.dma_start(out=w_sb, in_=wqkv.rearrange("(k p) f -> p k f", p=128))
    b_sb = consts.tile([128, 3], f32, tag="b")
    nc.sync.dma_start(out=b_sb, in_=bqkv)

    # --- warmup: ~10 N=512 matmuls (one accumulation chain so nothing is
    # dead) + a tiny exp to pull the ACT table load off the critical path.
    warm_ps = miscp.tile([128, SUB], f32, tag="misc", name="warm")
    for i in range(10):
        nc.tensor.matmul(warm_ps[:, :], id_sb[:, :], wrm_sb[:, :],
                         start=(i == 0), stop=(i == 9))
    warm_out = warmsb.tile([128, 528], bf16, tag="wo")
    nc.scalar.activation(warm_out[:, 512:528], id_sb[:, 0:16], AF.Exp)
    nc.vector.tensor_copy(warm_out[:, 0:512], warm_ps[:, :])
    nc.sync.dma_start(out=wsink, in_=warm_out)

    # early-attention deps go first on the scalar queue, before batch-1 x
    w16_sb = consts.tile([128, 8, 384], bf16, tag="w16")
    nc.scalar.dma_start(out=w16_sb,
                        in_=wqkv16.rearrange("(k p) f -> p k f", p=128))
    dmsk_sb = consts.tile([128, 2, 256], f8, tag="dmsk")
    nc.scalar.dma_start(out=dmsk_sb, in_=dmsk)

    # x^T: batch 0 on sync queue, batch 1 on scalar queue (parallel)
    xps, xes = [], []
    for b in range(B):
        eng = nc.sync if b == 0 else nc.scalar
        xp = xpool.tile([128, 8, T], f8 if QKV_FP8 else bf16, tag="xp",
                        name=f"xp{b}")
        xsrc = xT[b].rearrange("(j p) t -> p j t", p=128)
        xe = xpool.tile([128, 8, 256], bf16, tag="xe", name=f"xe{b}")
        eng.dma_start(out=xe, in_=xTe[b].rearrange(
            "(j p) t -> p j t", p=128))
        xes.append(xe)
        for tg in range(T // SUB):
            t0 = tg * SUB
            eng.dma_start(out=xp[:, :, t0:t0 + SUB],
                          in_=xsrc[:, :, t0:t0 + SUB])
        xps.append(xp)

    # later-needed consts after batch-0 x
    wp_sb = consts.tile([128, C], bf16, tag="wp")
    nc.sync.dma_start(out=wp_sb, in_=wp)

    filler = deque()
    fstate = {"cost": 0, "pops_left": 48}  # 48 attention chunk-pop sites

    def fpush(thunks):
        filler.extend(thunks)
        fstate["cost"] += sum(c for c, _ in thunks)

    def pop_filler(budget=None):
        # spread the remaining filler evenly over the remaining attention
        # chunks so the PE neither starves late nor hoards early
        if budget is None:
            left = max(1, fstate["pops_left"])
            budget = max(500, fstate["cost"] // left)
        while filler and budget > 0:
            cost, th = filler.popleft()
            fstate["cost"] -= cost
            th()
            budget -= cost

    def make_qkv(b):
        """qkvT tiles + thunks per (tg, m, n): 4-MM fp8-DR chains.

        Returns (dsts, front, rest): `front` covers tokens 0-1023 plus the
        bf16 early-token fix (everything q-tile 0 attention needs); `rest`
        is the tg=1 half, safe to run as attention filler."""
        dsts = [qkvpool.tile([128, T], bf16, tag="qkv", name=f"qkv{b}_{m}")
                for m in range(3)]
        by_tg = {0: [], 1: []}
        for tg in range(T // 1024):
            for m in range(3):
                for n in range(2):
                    def th(m=m, tg=tg, n=n):
                        t0 = tg * 1024 + n * SUB
                        pg = miscp.tile([128, SUB], f32, tag="misc",
                                        name="pg")
                        if QKV_FP8:
                            for c in range(4):
                                nc.tensor.matmul(
                                    pg[:, :],
                                    w_sb[:, 2 * c:2 * c + 2,
                                         128 * m:128 * m + 128],
                                    xps[b][:, 2 * c:2 * c + 2, t0:t0 + SUB],
                                    start=(c == 0), stop=(c == 3),
                                    perf_mode=DR,
                                )
                            nc.vector.tensor_scalar(
                                out=dsts[m][:, t0:t0 + SUB], in0=pg[:, :],
                                scalar1=1.0 / WSCL, scalar2=b_sb[:, m:m + 1],
                                op0=mybir.AluOpType.mult,
                                op1=mybir.AluOpType.add)
                            pass
                        else:
                            for kc in range(8):
                                nc.tensor.matmul(
                                    pg[:, :],
                                    w16_sb[:, kc, 128 * m:128 * m + 128],
                                    xps[b][:, kc, t0:t0 + SUB],
                                    start=(kc == 0), stop=(kc == 7),
                                )
                            nc.vector.tensor_scalar(
                                out=dsts[m][:, t0:t0 + SUB], in0=pg[:, :],
                                scalar1=b_sb[:, m:m + 1], scalar2=None,
                                op0=mybir.AluOpType.add)
                    by_tg[tg].append((1150, th))
        # tokens 0-255 recomputed in bf16: softmax rows with few valid keys
        # amplify fp8 noise, so the early tokens' q/k/v must be clean.
        fix = []
        if QKV_FP8:
            for m in range(3):
                def thfix(m=m):
                    pg = miscp.tile([128, 256], f32, tag="misc", name="pgf")
                    for kc in range(8):
                        nc.tensor.matmul(
                            pg[:, :],
                            w16_sb[:, kc, 128 * m:128 * m + 128],
                            xes[b][:, kc, :],
                            start=(kc == 0), stop=(kc == 7),
                        )
                    nc.vector.tensor_scalar(
                        out=dsts[m][:, 0:256], in0=pg[:, :],
                        scalar1=b_sb[:, m:m + 1], scalar2=None,
                        op0=mybir.AluOpType.add)
                fix.append((1000, thfix))
        return dsts, by_tg[0] + fix, by_tg[1]

    def make_vt(b, vT_t):
        """V to token-major [128, 16, 2, 72] fp8 with ones cols; 9 thunks.

        Chunks 0/1 (k < 256) are additionally kept in bf16 (vtb) for the
        precision-critical first O pair of q-tile 0."""
        vt = vtmpool.tile([128, 16, HPC, 72], f8 if O_FP8 else bf16,
                          tag="vtm", name=f"vt{b}")
        vtb = vtmpool.tile([128, 2, HPC, 72], bf16, tag="vtb", name=f"vtb{b}")

        def th0():
            nc.vector.memset(vt[:, :, :, 64:65], 1.0)
            nc.vector.memset(vtb[:, :, :, 64:65], 1.0)
        thunks = [(150, th0)]
        for j0 in range(0, T // 128, 2):
            def th(j0=j0):
                for j in (j0, j0 + 1):
                    tp = miscp.tile([128, 128], bf16, tag="misc", name="tp")
                    nc.tensor.transpose(
                        tp[:, :], vT_t[:, 128 * j:128 * j + 128], id_sb[:, :])
                    nc.vector.tensor_copy(
                        out=vt[:, j, :, 0:64],
                        in_=tp.rearrange("p (h c) -> p h c", h=HPC),
                    )
                    if j < 2:
                        nc.vector.tensor_copy(
                            out=vtb[:, j, :, 0:64],
                            in_=tp.rearrange("p (h c) -> p h c", h=HPC),
                        )
            thunks.append((650, th))
        return (vt, vtb), thunks

    def make_proj(b, q0, un, trange):
        """Projection thunks for q-rows trange of one q-tile."""
        thunks = []
        for ts in trange:
            for ct in range(C // SUB):
                def th(ts=ts, ct=ct):
                    a0 = q0 + ts * 128
                    pp = miscp.tile([128, SUB], f32, tag="misc", name="pp")
                    nc.tensor.matmul(
                        pp[:, :],
                        un[:, ts * 128:(ts + 1) * 128],
                        wp_sb[:, ct * SUB:(ct + 1) * SUB],
                        start=True, stop=True,
                    )
                    ob = outsb.tile([128, SUB], bf16, tag="osb")
                    nc.any.tensor_copy(ob[:, :], pp[:, :])
                    nc.sync.dma_start(
                        out=outp[b, a0:a0 + 128, ct * SUB:(ct + 1) * SUB],
                        in_=ob[:, :])
                thunks.append((450, th))
        return thunks

    # batch 0 front work: only what q-tile-0 attention needs runs densely
    # (QKV tokens 0-1023 + fix + V chunks 0-7); the rest becomes filler.
    qkv0, front0, rest0 = make_qkv(0)
    for _, th in front0:
        th()
    vt0, vth0 = make_vt(0, qkv0[2])
    for _, th in vth0[:5]:
        th()

    qkv_t, vt_t = {0: qkv0}, {0: vt0}

    for b in range(B):
        if b == 0:
            # queue the rest of the front work + batch 1 as attention filler
            qkv1, front1, rest1 = make_qkv(1)
            vt1, vth1 = make_vt(1, qkv1[2])
            fpush(rest0)
            fpush(vth0[5:])
            fpush(front1)
            fpush(rest1)
            fpush(vth1)
            qkv_t[1], vt_t[1] = qkv1, vt1
        qT_t, kT_t, vT_t = qkv_t[b]
        vt, vtb = vt_t[b]

        for qt in range(T // QT):
            q0 = qt * QT
            npair = (q0 + QT) // KP
            un = unormp.tile([128, QT], bf16, tag="un", name=f"un{b}{qt}")
            undone = [0, 0]  # per-half: heads whose norm is emitted
            def emit_o(h, ot, p, pt2):
                """O^T accumulate for k-pair p: fp8 DoubleRow, K=256."""
                lsp = max(0, p * KP - q0)
                diag = p * KP >= q0
                vpair = vt[:, 2 * p:2 * p + 2, h, 0:65]
                for n in range(QT // SUB):
                    s0 = max(n * SUB, lsp)
                    if s0 >= (n + 1) * SUB:
                        continue
                    if diag and s0 == lsp:
                        s0 = lsp + KP  # masked region emitted separately
                        if s0 >= (n + 1) * SUB:
                            continue
                    last_p = (q0 + (n + 1) * SUB) // KP - 1
                    nc.tensor.matmul(
                        ot[:, s0:(n + 1) * SUB],
                        vpair,
                        pt2[:, :, s0:(n + 1) * SUB],
                        start=(p == 0 and q0 > 0), stop=(p == last_p),
                        perf_mode=DR,
                    )
                if diag:
                    # region already started by p=0's full-subtile MM
                    n0 = lsp // SUB
                    last_p = (q0 + (n0 + 1) * SUB) // KP - 1
                    nc.tensor.matmul(
                        ot[:, lsp:lsp + KP],
                        vpair,
                        pt2[:, :, lsp:lsp + KP],
                        start=False, stop=(p == last_p),
                        perf_mode=DR,
                    )

            def emit_o_chunk(h, ot, kc, ptj, vsrc):
                """Per-chunk O^T accumulate (K=128, non-DR)."""
                ls = max(0, kc * KC - q0)
                diag = kc * KC >= q0
                for n in range(QT // SUB):
                    s0 = max(n * SUB, ls)
                    if s0 >= (n + 1) * SUB:
                        continue
                    if diag and s0 == ls:
                        s0 = ls + 128
                        if s0 >= (n + 1) * SUB:
                            continue
                    last_kc = (q0 + (n + 1) * SUB) // KC - 1
                    nc.tensor.matmul(
                        ot[:, s0:(n + 1) * SUB], vsrc,
                        ptj[:, s0:(n + 1) * SUB],
                        start=(kc == 0), stop=(kc == last_kc),
                    )
                if diag:
                    n0 = ls // SUB
                    last_kc = (q0 + (n0 + 1) * SUB) // KC - 1
                    nc.tensor.matmul(
                        ot[:, ls:ls + 128], vsrc, ptj[:, ls:ls + 128],
                        start=False, stop=(kc == last_kc),
                    )

            def norm_half(h, ot, half):
                """Normalize cols [half*SUB, (half+1)*SUB) of head h into
                un as soon as their O accumulation completes."""
                c0 = half * SUB
                se = rows.tile([1, SUB], f32, tag="se", name=f"se{h}")
                nc.vector.tensor_copy(se[:, :], ot[64:65, c0:c0 + SUB])
                rc = rows.tile([1, SUB], f32, tag="rc", name=f"rc{h}")
                nc.vector.reciprocal_approx_fast(rc[:, :], se[:, :])
                rb = rows.tile([64, SUB], f32, tag="rb", name=f"rb{h}")
                nc.gpsimd.partition_broadcast(rb[:, :], rc[:, :])
                nc.vector.tensor_mul(
                    un[64 * h:64 * h + 64, c0:c0 + SUB],
                    ot[0:64, c0:c0 + SUB], rb[:, :])
                undone[half] += 1
                if undone[half] == HPC:
                    # both heads done: this half's proj can go out
                    fpush(make_proj(
                        b, q0, un, range(4 * half, 4 * half + 4)))

            def emit_pair_o(h, ot, p, pt2):
                bf_pair = O_FP8 and (q0 == 0 and p == 0)
                if not O_FP8:
                    for j in range(2):
                        kc = 2 * p + j
                        emit_o_chunk(h, ot, kc, pt2[:, j], vt[:, kc, h, 0:65])
                elif bf_pair:
                    for j in range(2):
                        emit_o_chunk(h, ot, j, pt2[:, j], vtb[:, j, h, 0:65])
                else:
                    emit_o(h, ot, p, pt2)
                # completed column halves can normalize immediately
                if (q0 + (p + 1) * KP) % SUB == 0:
                    half = ((p + 1) * KP - q0) // SUB - 1
                    if 0 <= half < 2:
                        norm_half(h, ot, half)

            # S runs row-packed: both heads' K=64 matmuls go to disjoint
            # 64-row groups of the PE array (tile_position auto-derived from
            # base partitions) and execute concurrently. Head 0's O is
            # emitted inline; head 1's P^T tiles buffer in SBUF and its O
            # trails as natural PE backlog.
            ot0 = otp.tile([65, QT], f32, tag="ot", name="ot0")
            pts1 = []
            for p in range(npair):
                lsp = max(0, p * KP - q0)
                diag = p * KP >= q0
                bf_pair = O_FP8 and (q0 == 0 and p == 0)
                pt2s = []
                for h in range(HPC):
                    if bf_pair:
                        pt2s.append(ptbpool.tile([128, 2, QT], bf16,
                                                 tag="ptb", name=f"ptb{h}"))
                    else:
                        pt2s.append(ptpool.tile(
                            [128, 2, QT], f8 if O_FP8 else bf16, tag="pt",
                            name=f"pt{h}"))
                for j in range(2):  # the two k-chunks of the pair
                    kc = 2 * p + j
                    k0 = kc * KC
                    ls = max(0, k0 - q0)
                    for h in range(HPC):
                        st = stp.tile([128, QT], f32, tag="st", name=f"st{h}")
                        for n in range(QT // SUB):
                            s0 = max(n * SUB, ls)
                            if s0 >= (n + 1) * SUB:
                                continue
                            nc.tensor.matmul(
                                st[:, s0:(n + 1) * SUB],
                                kT_t[64 * h:64 * h + 64, k0:k0 + KC],
                                qT_t[64 * h:64 * h + 64,
                                     q0 + s0:q0 + (n + 1) * SUB],
                                start=True, stop=True,
                            )
                        nc.scalar.activation(
                            pt2s[h][:, j, ls:QT], st[:, ls:QT], AF.Exp,
                            scale=SCALE)
                    if not (b == 0 and qt == 0 and p == 0):
                        pop_filler()
                    fstate["pops_left"] -= 1
                if diag:
                    for h in range(HPC):
                        nc.vector.memset(pt2s[h][:, 1, lsp:lsp + 128], 0.0)
                        nc.vector.tensor_mul(
                            pt2s[h][:, :, lsp:lsp + KP],
                            pt2s[h][:, :, lsp:lsp + KP], dmsk_sb[:, :, :])
                emit_pair_o(0, ot0, p, pt2s[0])
                pts1.append(pt2s[1])
            ot1 = otp.tile([65, QT], f32, tag="ot", name="ot1")
            for p in range(npair):
                emit_pair_o(1, ot1, p, pts1[p])

    pop_filler(10**9)



def build():
    if "nc" in _CACHE:
        return _CACHE["nc"]
    nc = bacc.Bacc("TRN2", target_bir_lowering=False, debug=False,
                   num_devices=NCORES)
    with tile.TileContext(nc) as tc:
        _emit(tc)
    nc.compile()
    _CACHE["nc"] = nc
    return nc


def make_in_maps(x, qkv_w, qkv_b, proj_w):
    import ml_dtypes
    bf16 = ml_dtypes.bfloat16
    f8 = ml_dtypes.float8_e4m3
    x = np.asarray(x, dtype=np.float32)
    qkv_w = np.asarray(qkv_w, dtype=np.float32)
    qkv_b = np.asarray(qkv_b, dtype=np.float32)
    proj_w = np.asarray(proj_w, dtype=np.float32)

    xTf = np.ascontiguousarray(x.transpose(0, 2, 1))
    xT = xTf.astype(f8 if QKV_FP8 else bf16)
    xTe = np.ascontiguousarray(xTf[:, :, 0:256]).astype(bf16)
    # diag-pair mask [128, 2, 256]: even chunk = [tril | ones],
    # odd chunk = [zeros | tril]
    tri = (np.arange(128)[None, :] >= np.arange(128)[:, None])
    dmsk = np.zeros((128, 2, 256), dtype=np.float32)
    dmsk[:, 0, 0:128] = tri
    dmsk[:, 0, 128:256] = 1.0
    dmsk[:, 1, 128:256] = tri
    dmsk = dmsk.astype(f8)
    ident = np.eye(128, dtype=bf16)
    wrm = np.zeros((128, 512), dtype=bf16)

    in_maps = []
    for c in range(NCORES):
        s = 64 * HPC * c  # first feature row of this core's heads
        wq = qkv_w[:, s:s + 128]
        wk = qkv_w[:, C + s:C + s + 128]
        wv = qkv_w[:, 2 * C + s:2 * C + s + 128]
        wqkv_cat = np.concatenate([wq, wk, wv], axis=1)
        wqkv_c = np.ascontiguousarray(wqkv_cat * WSCL).astype(f8)
        wqkv16_c = np.ascontiguousarray(wqkv_cat).astype(bf16)
        bqkv_c = np.ascontiguousarray(np.stack(
            [qkv_b[s:s + 128], qkv_b[C + s:C + s + 128],
             qkv_b[2 * C + s:2 * C + s + 128]], axis=1))
        wp_c = np.ascontiguousarray(proj_w[s:s + 128, :]).astype(bf16)
        in_maps.append({
            "xT": xT, "xTe": xTe, "wqkv": wqkv_c, "wqkv16": wqkv16_c,
            "bqkv": bqkv_c, "wp": wp_c,
            "dmsk": dmsk, "ident": ident, "wrm": wrm,
        })
    return in_maps


def kernel(x, qkv_w, qkv_b, proj_w, proj_b, _trace=False):
    nc = build()
    in_maps = make_in_maps(x, qkv_w, qkv_b, proj_w)
    res = run_bass_kernel_spmd(nc, in_maps, core_ids=list(range(NCORES)),
                               trace=_trace)
    acc = np.zeros((B, T, C), dtype=np.float64)
    for c in range(NCORES):
        acc += np.asarray(res.results[c]["outp"]).astype(np.float64)
    acc += np.asarray(proj_b, dtype=np.float64)
    out = acc.astype(np.float32)
    _CACHE["last_results"] = res
    return out
